# revision 14
# baseline (speedup 1.0000x reference)
"""Trainium2 Bass kernel: cached causal self-attention (dense transformer block).

Full module: y = CausalAttn(x; Wq, Wk, Wv) @ Wo.T + bo with
  B=4, S=2048, E=2048, H=16 heads, Dh=128, fp32 inputs.

Distribution: 8-way tensor parallel over heads (2 heads per NeuronCore).
Each core computes Q/K/V projections for its 2 heads (contraction over the
full embedding dim), causal-softmax attention for those heads, and a partial
output projection y_c = ctx_c @ Wo[:, c*256:(c+1)*256].T.  The host sums the
8 partials and adds the bias (the cross-head reduction of the output
projection), avoiding on-device collectives.

All matmul operands are bfloat16 (accumulation in fp32 PSUM): same 1
cycle/row PE rate as float32r at >=256-wide moving operands, but half the
HBM traffic (x in / y out dominate: 67 MB each per core) and half the
LDWEIGHTS occupancy on the PE.  The ~0.1% rounding is far inside the 2e-2
correctness budget.

Engine balance (the attention+projection phase is otherwise bound by the
Activation engine, whose exp eviction of each 512-wide score strip takes
slightly longer than the PE's three matmuls per strip):
  - scores are computed transposed (sT[k, q]) so exp(sT) feeds the attn@V
    matmul as the moving operand directly, no on-chip transpose.
  - softmax denominators (column sums of exp) come from a ones-vector
    matmul in a [1, CH] PSUM row; a single-op approximate reciprocal (DVE)
    frees the PSUM row quickly, GpSimd broadcasts it across partitions, and
    the DVE multiplies it into ctx at the attn@V PSUM eviction -- ctx
    enters the output projection normalized, so the two heads' partials
    share one PSUM accumulation chain.
  - y PSUM evictions ride on the DVE (tensor_copy), keeping the Activation
    engine exp-only during attention.
  - output-projection groups are interleaved one-per-k-tile into the
    attention loop (and the previous batch's last chunk into the QKV
    phase), so the PE fills exp-latency stalls with projection matmuls and
    the y write-out DMA spreads evenly.
"""

import math

import ml_dtypes
import numpy as np

import concourse.bacc as bacc
import concourse.mybir as mybir
import concourse.tile as tile
from concourse.bass_utils import run_bass_kernel_spmd

F32 = mybir.dt.float32
BF16 = mybir.dt.bfloat16
AF = mybir.ActivationFunctionType
ALU = mybir.AluOpType

NEG = -1.0e30

# Full-problem constants
EMB = 2048
N_HEADS = 16
HEAD_DIM = 128
B_FULL = 4
S_FULL = 2048
N_CORES = 8
HPC = N_HEADS // N_CORES  # heads per core = 2


def build(B=B_FULL, S=S_FULL, E=EMB, hpc=HPC, DH=HEAD_DIM, CH=512):
    """Build the per-core Bass program (same program on all 8 cores)."""
    SB = B * S
    DHC = hpc * DH          # per-core head dims (256)
    NE = E // 128           # e-tiles (contraction tiles)
    NCH = S // CH           # 512-wide chunks per sequence
    KPC = CH // 128         # k-tiles per chunk (4)
    NST = S // 128          # 128-row s-tiles per sequence
    NOC = E // CH           # output chunks
    scale = 1.0 / math.sqrt(DH)

    nc = bacc.Bacc("TRN2", target_bir_lowering=False, debug=False,
                   num_devices=N_CORES)

    xT = nc.dram_tensor("xT", [E, SB], BF16, kind="ExternalInput")
    wqT = nc.dram_tensor("wqT", [E, DHC], BF16, kind="ExternalInput")
    wkT = nc.dram_tensor("wkT", [E, DHC], BF16, kind="ExternalInput")
    wvT = nc.dram_tensor("wvT", [E, DHC], BF16, kind="ExternalInput")
    woT = nc.dram_tensor("woT", [DHC, E], BF16, kind="ExternalInput")
    masks = nc.dram_tensor("masks", [128, CH], BF16, kind="ExternalInput")
    ones = nc.dram_tensor("ones", [128, 1], BF16, kind="ExternalInput")
    y = nc.dram_tensor("y", [SB, E], BF16, kind="ExternalOutput")

    with tile.TileContext(nc) as tc:
        with (
            tc.tile_pool(name="wpool", bufs=1) as wpool,
            tc.tile_pool(name="xtp", bufs=3) as xtp,
            tc.tile_pool(name="qkv", bufs=1) as qkv,
            tc.tile_pool(name="ctxp", bufs=2) as ctxp,
            tc.tile_pool(name="expp", bufs=3) as expp,
            tc.tile_pool(name="denp", bufs=2) as denp_sb,
            tc.tile_pool(name="yout", bufs=3) as yout,
            tc.tile_pool(name="ps_mm", bufs=2, space="PSUM") as ps_mm,
            tc.tile_pool(name="ps_proj", bufs=2, space="PSUM") as ps_proj,
            tc.tile_pool(name="ps_av", bufs=1, space="PSUM") as ps_av,
            tc.tile_pool(name="ps_den", bufs=1, space="PSUM") as ps_den,
        ):
            # Resident weights / constants.  The first Q chain consumes
            # e-tiles in order: land the low wq/x quarters first so the PE
            # starts as soon as possible.
            wq_sb = wpool.tile([128, NE, DHC], BF16, tag="wq")
            wk_sb = wpool.tile([128, NE, DHC], BF16, tag="wk")
            wv_sb = wpool.tile([128, NE, DHC], BF16, tag="wv")
            wo_sb = wpool.tile([128, hpc, E], BF16, tag="wo")
            xT_r = xT.rearrange("(t p) s -> p t s", p=128)
            NEH = NE // 2
            NEQ = NE // 4
            wq_r = wqT.rearrange("(t p) d -> p t d", p=128)
            wk_r = wkT.rearrange("(t p) d -> p t d", p=128)
            x0a = xtp.tile([128, NEH, CH], BF16, tag="xta", name="x0a")
            x0b = xtp.tile([128, NEH, CH], BF16, tag="xtb", name="x0b")
            NE8 = NE // 8
            nc.sync.dma_start(wq_sb[:, 0:NE8, :], wq_r[:, 0:NE8, :])
            nc.sync.dma_start(x0a[:, 0:NE8, :], xT_r[:, 0:NE8, 0:CH])
            nc.sync.dma_start(wq_sb[:, NE8:NEQ, :], wq_r[:, NE8:NEQ, :])
            nc.sync.dma_start(x0a[:, NE8:NEQ, :], xT_r[:, NE8:NEQ, 0:CH])
            nc.sync.dma_start(wq_sb[:, NEQ:NEH, :], wq_r[:, NEQ:NEH, :])
            nc.sync.dma_start(x0a[:, NEQ:NEH, :], xT_r[:, NEQ:NEH, 0:CH])
            nc.sync.dma_start(x0b[:], xT_r[:, NEH:NE, 0:CH])
            nc.sync.dma_start(wq_sb[:, NEH:NE, :], wq_r[:, NEH:NE, :])
            nc.sync.dma_start(wk_sb[:, 0:NEH, :], wk_r[:, 0:NEH, :])
            nc.sync.dma_start(wk_sb[:, NEH:NE, :], wk_r[:, NEH:NE, :])
            nc.sync.dma_start(wv_sb[:], wvT.rearrange("(t p) d -> p t d", p=128))
            x1a = xtp.tile([128, NEH, CH], BF16, tag="xta", name="x1a")
            nc.sync.dma_start(x1a[:], xT_r[:, 0:NEH, CH:2 * CH])
            x1b = xtp.tile([128, NEH, CH], BF16, tag="xtb", name="x1b")
            nc.sync.dma_start(x1b[:], xT_r[:, NEH:NE, CH:2 * CH])
            xpre0 = {(0, 0): (x0a, x0b), (0, 1): (x1a, x1b)}
            nc.sync.dma_start(wo_sb[:], woT.rearrange("(h p) e -> p h e", p=128))
            mask_sb = wpool.tile([128, CH], BF16, tag="mask")
            nc.sync.dma_start(mask_sb[:], masks[:, :])
            ones_sb = wpool.tile([128, 1], BF16, tag="ones")
            nc.sync.dma_start(ones_sb[:], ones[:, :])

            def emit_one_proj(pctx, ps0, st, oc, evict="vector"):
                o0 = oc * CH
                p0 = ps_proj.tile([128, CH], F32, tag="proj")
                nc.tensor.matmul(
                    p0[:], pctx[:, 0, st * 128:(st + 1) * 128],
                    wo_sb[:, 0, o0:o0 + CH], start=True, stop=False)
                nc.tensor.matmul(
                    p0[:], pctx[:, 1, st * 128:(st + 1) * 128],
                    wo_sb[:, 1, o0:o0 + CH], start=False, stop=True)
                ysb = yout.tile([128, CH], BF16, tag="ysb")
                if evict == "vector":
                    # during attention the Activation engine is saturated
                    # with exp; evict on the DVE
                    nc.vector.tensor_copy(ysb[:], p0[:])
                elif evict == "split":
                    # final flush: halve the eviction latency by running the
                    # two halves on the (then idle) Activation engine + DVE
                    nc.scalar.activation(ysb[:, 0:CH // 2], p0[:, 0:CH // 2],
                                         AF.Identity)
                    nc.vector.tensor_copy(ysb[:, CH // 2:], p0[:, CH // 2:])
                else:
                    # during QKV the Activation engine is nearly idle
                    nc.scalar.activation(ysb[:], p0[:], AF.Identity)
                nc.gpsimd.dma_start(
                    y[ps0 + st * 128:ps0 + (st + 1) * 128, o0:o0 + CH],
                    ysb[:])

            def proj_groups(g):
                return [(st, oc) for st in range(g * KPC, (g + 1) * KPC)
                        for oc in range(NOC)]

            # pending: (ctx tile, batch row offset, remaining (st, oc) list)
            pending = None
            for b in range(B):
                s0 = b * S
                # ---------------- Phase A: Q/K/V projections -------------
                # The previous batch's last attention chunk still owes its
                # output projection; its groups are drip-fed between this
                # phase's QKV chunks (the PE is dense here but the other
                # engines are nearly idle, and it spreads the y DMA).
                qT = qkv.tile([128, hpc, S], BF16, tag="qT")
                kT = qkv.tile([128, hpc, S], BF16, tag="kT")
                v_sb = qkv.tile([128, NST, DHC], BF16, tag="v")
                if b == 0:
                    xpre = dict(xpre0)
                for ch in range(NCH):
                    c0 = ch * CH
                    xta, xtb = xpre.pop((b, ch))
                    # prefetch TWO chunks ahead: the transfer (~12us) plus
                    # its issue latency fits well inside two chunk periods,
                    # so phase A never waits on x again
                    gi = b * NCH + ch + 2
                    if gi < B * NCH:
                        nb_, nch = divmod(gi, NCH)
                        n0 = nb_ * S + nch * CH
                        xna = xtp.tile([128, NEH, CH], BF16, tag="xta",
                                       name="xna")
                        nc.sync.dma_start(xna[:], xT_r[:, 0:NEH, n0:n0 + CH])
                        xnb = xtp.tile([128, NEH, CH], BF16, tag="xtb",
                                       name="xnb")
                        nc.sync.dma_start(xnb[:], xT_r[:, NEH:NE, n0:n0 + CH])
                        xpre[(nb_, nch)] = (xna, xnb)

                    def xslice(et, lo=None, hi=None):
                        t = xta if et < NEH else xtb
                        e = et if et < NEH else et - NEH
                        if lo is None:
                            return t[:, e, :]
                        return t[:, e, lo:hi]

                    # Both heads' Q (resp. K) chains accumulate into the
                    # two halves of one 2-bank PSUM tile and evict in a
                    # single batched activation -- 12 evictions per chunk
                    # become 3, amortizing the Activation engine's fixed
                    # per-op cost.
                    qp = ps_mm.tile([128, hpc, CH], F32, tag="qkvp")
                    for h in range(hpc):
                        for et in range(NE):
                            nc.tensor.matmul(
                                qp[:, h, :], wq_sb[:, et, h * DH:(h + 1) * DH],
                                xslice(et),
                                start=(et == 0), stop=(et == NE - 1))
                    nc.scalar.activation(qT[:, 0:hpc, c0:c0 + CH], qp[:],
                                         AF.Identity, scale=scale)
                    kp = ps_mm.tile([128, hpc, CH], F32, tag="qkvp")
                    for h in range(hpc):
                        for et in range(NE):
                            nc.tensor.matmul(
                                kp[:, h, :], wk_sb[:, et, h * DH:(h + 1) * DH],
                                xslice(et),
                                start=(et == 0), stop=(et == NE - 1))
                    nc.scalar.activation(kT[:, 0:hpc, c0:c0 + CH], kp[:],
                                         AF.Identity)
                    vp = ps_mm.tile([128, KPC, DHC], F32, tag="qkvp")
                    for st in range(KPC):
                        for et in range(NE):
                            nc.tensor.matmul(
                                vp[:, st, :], xslice(et, st * 128, (st + 1) * 128),
                                wv_sb[:, et, :],
                                start=(et == 0), stop=(et == NE - 1))
                    nc.scalar.activation(v_sb[:, ch * KPC:(ch + 1) * KPC, :],
                                         vp[:], AF.Identity)
                    if pending is not None:
                        pctx, ps0, grps = pending
                        for st_, oc_ in grps[:NOC]:
                            emit_one_proj(pctx, ps0, st_, oc_, evict="scalar")
                        grps = grps[NOC:]
                        pending = (pctx, ps0, grps) if grps else None
                if pending is not None:
                    pctx, ps0, grps = pending
                    for st_, oc_ in grps:
                        emit_one_proj(pctx, ps0, st_, oc_, evict="scalar")
                    pending = None

                # ------- Phase B+C: attention with interleaved projection ----
                # Per 512-chunk g: both heads' attention for queries in g,
                # softmax normalization folded into the ctx eviction, with
                # chunk g-1's output projection drip-fed one group per
                # k-tile (the PE covers exp latency with projection work and
                # the y DMA spreads across the window).
                ctxT = ctxp.tile([128, hpc, S], BF16, tag="ctxT")
                for g in range(NCH):
                    grps = proj_groups(g - 1) if g > 0 else []
                    for h in range(hpc):
                        nk = KPC * (g + 1)
                        avp = ps_av.tile([128, CH], F32, tag="av")
                        dnp = ps_den.tile([1, CH], F32, tag="den")
                        def kt_off(kt):
                            j = kt - (nk - KPC)
                            return 128 * j if j > 0 else 0

                        exs = {}
                        def emit_av(kt):
                            ex, i = exs[kt]
                            off = kt_off(kt)
                            nc.tensor.matmul(
                                avp[:, off:],
                                v_sb[:, kt, h * DH:(h + 1) * DH],
                                ex[:, i, off:],
                                start=(kt == 0), stop=(kt == nk - 1),
                                skip_group_check=True)

                        def emit_den(kt):
                            ex, i = exs[kt]
                            off = kt_off(kt)
                            nc.tensor.matmul(
                                dnp[:, off:], ones_sb[:], ex[:, i, off:],
                                start=(kt == 0), stop=(kt == nk - 1),
                                skip_group_check=True)

                        # k-tiles are processed in pairs sharing one 2-bank
                        # PSUM tile and (where the diagonal suffix allows)
                        # one batched exp over both strips, halving the
                        # Activation engine's fixed per-op cost.  Columns of
                        # a diagonal strip below its suffix offset hold
                        # unwritten PSUM garbage; their exp outputs are
                        # never read.  AV/den matmuls for a pair are emitted
                        # one pair late: their kt==0 group-starts must wait
                        # for the previous head's avp/dnp reads, and the
                        # delay hides that latency behind real work instead
                        # of stalling the PE in-order.  den before AV so the
                        # longer reciprocal->broadcast->multiply chain of
                        # the final pair starts first.
                        for p in range(nk // 2):
                            k0, k1 = 2 * p, 2 * p + 1
                            sp = ps_mm.tile([128, 2, CH], F32, tag="qkvp",
                                            name="sp")
                            for i, kt in ((0, k0), (1, k1)):
                                off = kt_off(kt)
                                nc.tensor.matmul(
                                    sp[:, i, off:],
                                    kT[:, h, kt * 128:(kt + 1) * 128],
                                    qT[:, h, g * CH + off:(g + 1) * CH],
                                    start=True, stop=True)
                                if kt >= nk - KPC:
                                    # mask col c: masked iff c < p
                                    nc.vector.tensor_add(
                                        sp[:, i, off:], sp[:, i, off:],
                                        mask_sb[:, 0:CH - off])
                            ex = expp.tile([128, 2, CH], BF16, tag="ex")
                            if kt_off(k1) <= 128:
                                nc.scalar.activation(ex[:], sp[:], AF.Exp)
                            else:
                                for i, kt in ((0, k0), (1, k1)):
                                    off = kt_off(kt)
                                    nc.scalar.activation(ex[:, i, off:],
                                                         sp[:, i, off:],
                                                         AF.Exp)
                            exs[k0] = (ex, 0)
                            exs[k1] = (ex, 1)
                            if p > 0:
                                emit_den(k0 - 2)
                                emit_den(k1 - 2)
                                emit_av(k0 - 2)
                                emit_av(k1 - 2)
                            if grps:
                                st_, oc_ = grps.pop(0)
                                emit_one_proj(ctxT, s0, st_, oc_)
                            if grps:
                                st_, oc_ = grps.pop(0)
                                emit_one_proj(ctxT, s0, st_, oc_)
                        emit_den(nk - 2)
                        emit_den(nk - 1)
                        emit_av(nk - 2)
                        emit_av(nk - 1)
                        # 1/denominator row: single-op approximate reciprocal
                        # (~18 correct bits -- den is in [1e2, 4e3], far from
                        # the undefined edge cases) frees dnp for the next
                        # chunk quickly; GpSimd broadcasts it across
                        # partitions; the DVE multiplies it into ctx at the
                        # attn@V PSUM eviction.
                        rrow = denp_sb.tile([1, CH], F32, tag="rrow")
                        nc.vector.reciprocal_approx_fast(rrow[:], dnp[:])
                        rden = denp_sb.tile([128, CH], F32, tag="rden")
                        nc.gpsimd.partition_broadcast(rden[:], rrow[:])
                        nc.vector.tensor_mul(ctxT[:, h, g * CH:(g + 1) * CH],
                                             avp[:], rden[:])
                    for st_, oc_ in grps:
                        emit_one_proj(ctxT, s0, st_, oc_)
                pending = (ctxT, s0, proj_groups(NCH - 1))
            if pending is not None:
                pctx, ps0, grps = pending
                for st_, oc_ in grps:
                    emit_one_proj(pctx, ps0, st_, oc_, evict="split")
                pending = None
    nc.finalize()
    return nc


def host_consts(S=S_FULL, CH=512):
    """Mask / ones constant inputs."""
    p = np.arange(128)[:, None]
    c = np.arange(CH)[None, :]
    # strict lower triangle: masked iff c < p (diagonal k-tile suffix mask)
    masks = np.where(c < p, np.float32(NEG), np.float32(0.0))
    masks = np.ascontiguousarray(masks.astype(ml_dtypes.bfloat16))
    return {
        "masks": masks,
        "ones": np.ones((128, 1), dtype=ml_dtypes.bfloat16),
    }


def _bf16(a):
    return np.ascontiguousarray(a.astype(ml_dtypes.bfloat16))


def host_inputs(x, Wq, Wk, Wv, Wo, B=B_FULL, S=S_FULL, E=EMB, hpc=HPC,
                DH=HEAD_DIM, CH=512):
    """Shard + lay out the full inputs for the 8 cores."""
    SB = B * S
    DHC = hpc * DH
    xT = _bf16(x.reshape(SB, E).T)
    consts = host_consts(S, CH)

    in_maps = []
    for c in range(N_CORES):
        lo, hi = c * DHC, (c + 1) * DHC
        in_maps.append({
            "xT": xT,
            "wqT": _bf16(Wq[lo:hi, :].T),
            "wkT": _bf16(Wk[lo:hi, :].T),
            "wvT": _bf16(Wv[lo:hi, :].T),
            "woT": _bf16(Wo[:, lo:hi].T),
            **consts,
        })
    return in_maps


def kernel(x, Wq, Wk, Wv, Wo, bo):
    x = np.asarray(x, dtype=np.float32)
    Wq = np.asarray(Wq, dtype=np.float32)
    Wk = np.asarray(Wk, dtype=np.float32)
    Wv = np.asarray(Wv, dtype=np.float32)
    Wo = np.asarray(Wo, dtype=np.float32)
    bo = np.asarray(bo, dtype=np.float32)

    nc = build()
    in_maps = host_inputs(x, Wq, Wk, Wv, Wo)
    res = run_bass_kernel_spmd(nc, in_maps, list(range(N_CORES)))
    y = res.results[0]["y"].astype(np.float64)
    for c in range(1, N_CORES):
        y += res.results[c]["y"].astype(np.float64)
    y = (y + bo).astype(np.float32)
    return y.reshape(B_FULL, S_FULL, EMB)
